# revision 29
# baseline (speedup 1.0000x reference)
"""CMA (chunked-memory attention) Trainium2 kernel.

Sharding: 8 cores = batch(2) x head-groups(4). Each core computes 4 heads
of one batch element end-to-end (q/k/v projections, attention, and its
partial contribution Y_hs @ Wo[hs,:] to the output projection). The host
pre-transposes inputs into [C, S] layout, folds the 1/sqrt(D) attention
scale into Wq, evaluates the tiny gate path g = sigmoid(x @ (Wq@gate_W) + b)
(0.1% of FLOPs), and sum-reduces the four per-head-group output partials
per batch at unshard time (the reduction implied by splitting Wo rows).

Per-core kernel structure:
  phase 1: stream kvT column blocks [C, 512]; project kT/qT (transposed,
           head-channel on partitions) and v (s on partitions, with a
           fused ones-column per head for the softmax denominator).
  phase 2: per head-pair, per 512-wide t strip: QK^T with both heads
           row-packed in the 128x128 PE array (K=64 each), exp straight
           out of PSUM on ScalarE (scores are in [-3, 3], so no
           max-subtraction is needed), causal handling via partial-column
           matmuls plus one triangular [128,128] mask on diagonal blocks,
           then w^T @ [v | 1] accumulating numerator and denominator in
           one PSUM tile. Local (causal chunk) and memory key ranges use
           the same accumulator sequentially; the gate blends them in a
           small DVE epilogue that also normalizes.
  phase 3: out = Y^T.T @ Wo[hs,:] accumulated over the two 128-row
           head-channel tiles.
"""

import numpy as np

# ---------------------------------------------------------------- config

N_CORES = 8
B, T, MEM, C, H = 2, 2048, 512, 1024, 16
SM = 2 * MEM          # total memory keys
S = T + SM            # total keys
HL = H // (N_CORES // B)   # 4 local heads per core
D = C // H            # 64
CC = HL * D           # 256 local head channels
NK = C // 128         # contraction k-tiles
CB = 512              # kvT column-block width
NB = S // CB          # 6 column blocks
NJ = T // 512         # 4 t-strips
NLT = T // 128        # 16 local s-tiles
NMT = SM // 128       # 8 memory s-tiles

GATE_REG_STRENGTH = 0.01

_CACHE = {}


# ------------------------------------------------------------ bass build

def _build(mm_dtype="float32r", reps=1):
    """Build + compile the per-core Bass program (identical on all cores)."""
    import concourse.bass as bass
    import concourse.tile as tile
    from concourse import bacc, mybir

    f32 = mybir.dt.float32
    bf16 = mybir.dt.bfloat16
    if mm_dtype == "bfloat16":
        SD = bf16          # storage dtype for matmul operands
    elif mm_dtype == "float32r":
        # walrus requires fp32r matmul operands to be *produced* as fp32r
        # (the writing engine rounds), so the whole storage path is fp32r.
        SD = mybir.dt.float32r
    else:
        SD = f32

    def mmc(ap):
        return ap

    nc = bacc.Bacc("TRN2", target_bir_lowering=False, debug=False,
                   num_devices=N_CORES)

    kvT = nc.dram_tensor("kvT", [C, S], SD, kind="ExternalInput").ap()
    wq = nc.dram_tensor("wq", [C, CC], SD, kind="ExternalInput").ap()
    wk = nc.dram_tensor("wk", [C, CC], SD, kind="ExternalInput").ap()
    wv = nc.dram_tensor("wv", [C, CC], SD, kind="ExternalInput").ap()
    wo = nc.dram_tensor("wo", [CC, C], SD, kind="ExternalInput").ap()
    gR = nc.dram_tensor("gR", [HL * D, T], f32, kind="ExternalInput").ap()
    onec = nc.dram_tensor("onec", [1, 64], SD, kind="ExternalInput").ap()
    trineg = nc.dram_tensor("trineg", [128, 128], bf16,
                            kind="ExternalInput").ap()
    ident = nc.dram_tensor("ident", [128, 128], bf16,
                           kind="ExternalInput").ap()
    outp = nc.dram_tensor("outp", [T, C], f32, kind="ExternalOutput").ap()

    Exp = mybir.ActivationFunctionType.Exp

    with tile.TileContext(nc) as tc:
        from contextlib import ExitStack
        with ExitStack() as ctx:
            const = ctx.enter_context(tc.tile_pool(name="const", bufs=1))
            kvp = ctx.enter_context(tc.tile_pool(name="kvp", bufs=2))
            qtp = ctx.enter_context(tc.tile_pool(name="qtp", bufs=1))
            ktp = ctx.enter_context(tc.tile_pool(name="ktp", bufs=1))
            vp = ctx.enter_context(tc.tile_pool(name="vp", bufs=1))
            ytp = ctx.enter_context(tc.tile_pool(name="ytp", bufs=1))
            wtp = ctx.enter_context(tc.tile_pool(name="wtp", bufs=2))
            ycp = ctx.enter_context(tc.tile_pool(name="ycp", bufs=1))
            epp = ctx.enter_context(tc.tile_pool(name="epp", bufs=2))
            osp = ctx.enter_context(tc.tile_pool(name="osp", bufs=2))
            pproj = ctx.enter_context(
                tc.tile_pool(name="pproj", bufs=2, space="PSUM"))
            pscp = ctx.enter_context(
                tc.tile_pool(name="pscp", bufs=2, space="PSUM"))
            pytp = ctx.enter_context(
                tc.tile_pool(name="pytp", bufs=2, space="PSUM"))

            # ---- constants (loaded once, outside the reps loop) ----
            wq_sb = [const.tile([128, CC], SD, name=f"wq{k}", tag=f"wq{k}") for k in range(NK)]
            wk_sb = [const.tile([128, CC], SD, name=f"wk{k}", tag=f"wk{k}") for k in range(NK)]
            wv_sb = [const.tile([128, CC], SD, name=f"wv{k}", tag=f"wv{k}") for k in range(NK)]
            wo_sb = [const.tile([128, C], SD, name=f"wo{k}", tag=f"wo{k}") for k in range(CC // 128)]
            tm_sb = const.tile([128, 128], bf16, name="tm", tag="tm")
            id_sb = const.tile([128, 128], bf16, name="idn", tag="idn")
            oc_sb = const.tile([1, 64], SD, name="oc", tag="oc")
            nc.scalar.dma_start(oc_sb[:], onec[:])
            for k in range(NK):
                nc.scalar.dma_start(wk_sb[k][:], wk[128 * k:128 * k + 128, :])
            for k in range(NK):
                nc.scalar.dma_start(wq_sb[k][:], wq[128 * k:128 * k + 128, :])
            for k in range(NK):
                nc.scalar.dma_start(wv_sb[k][:], wv[128 * k:128 * k + 128, :])
            for k in range(CC // 128):
                nc.scalar.dma_start(wo_sb[k][:], wo[128 * k:128 * k + 128, :])
            nc.scalar.dma_start(tm_sb[:], trineg[:])
            nc.scalar.dma_start(id_sb[:], ident[:])
            # warm the Exp table while phase 1 runs
            warm = const.tile([1, 1], f32, name="warm", tag="warm")
            nc.vector.memset(warm[:], 0.0)
            nc.scalar.activation(warm[:], warm[:], Exp)
            ones4 = const.tile([128, HL], f32, name="ones4", tag="ones4")
            nc.vector.memset(ones4[:], 1.0)

            def body(_iv=None):
                # persistent per-iteration outputs
                qt_sb = [qtp.tile([128, T], SD, name=f"qt{m}", tag=f"qt{m}") for m in range(2)]
                kt_sb = [ktp.tile([128, S], SD, name=f"kt{m}", tag=f"kt{m}") for m in range(2)]
                v_sb = [vp.tile([128, HL * (D + 1)], SD, name=f"v{si}", tag=f"v{si}")
                        for si in range(S // 128)]
                yt_sb = [ytp.tile([128, T], SD, name=f"yt{p}", tag=f"yt{p}") for p in range(2)]

                # ---------------- phase 1: projections ----------------
                for b in range(NB):
                    c0 = CB * b
                    kvt = [kvp.tile([128, CB], SD, name=f"kv{k}", tag=f"kv{k}")
                           for k in range(NK)]
                    for k in range(NK):
                        eng = nc.sync if k % 2 == 0 else nc.scalar
                        eng.dma_start(
                            kvt[k][:], kvT[128 * k:128 * k + 128, c0:c0 + CB])
                    # kT (and qT for local columns): [cout 128, cols]
                    for m in range(2):
                        pk = pproj.tile([128, CB], f32, name="pproj", tag="pproj")
                        for k in range(NK):
                            nc.tensor.matmul(
                                pk[:], mmc(wk_sb[k][:, 128 * m:128 * m + 128]),
                                mmc(kvt[k][:]), start=(k == 0), stop=(k == NK - 1))
                        nc.vector.tensor_copy(kt_sb[m][:, c0:c0 + CB], pk[:])
                    if c0 < T:
                        for m in range(2):
                            pq = pproj.tile([128, CB], f32, name="pproj", tag="pproj")
                            for k in range(NK):
                                nc.tensor.matmul(
                                    pq[:], mmc(wq_sb[k][:, 128 * m:128 * m + 128]),
                                    mmc(kvt[k][:]), start=(k == 0),
                                    stop=(k == NK - 1))
                            nc.vector.tensor_copy(qt_sb[m][:, c0:c0 + CB], pq[:])
                    # v: [s 128, cout CC] -> packed [s, HL*(D+1)] with ones col
                    for sv in range(CB // 128):
                        si = (CB * b) // 128 + sv
                        pv = pproj.tile([128, CC], f32, name="pproj", tag="pproj")
                        for k in range(NK):
                            nc.tensor.matmul(
                                pv[:],
                                mmc(kvt[k][:, 128 * sv:128 * sv + 128]),
                                mmc(wv_sb[k][:]), start=(k == 0),
                                stop=(k == NK - 1))
                        vt = v_sb[si]
                        # strided copy: head h -> cols [h*(D+1), h*(D+1)+D)
                        dst = vt[:].rearrange("p (h e) -> p h e", h=HL)[:, :, 0:D]
                        src = pv[:].rearrange("p (h d) -> p h d", h=HL)
                        nc.vector.tensor_copy(dst, src)
                        ones = vt[:].rearrange("p (h e) -> p h e", h=HL)[:, :, D:D + 1]
                        nc.vector.tensor_copy(
                            ones, ones4[:].unsqueeze(2))

                # -------- phase 2: attention + interleaved out-proj --------
                W3 = min(512, C)

                def do_strip(p, j):
                        t0 = 512 * j
                        pyt = [pytp.tile([D + 1, 512], f32, name="pyt", tag="pyt")
                               for h in range(2)]
                        gr = [epp.tile([D, 512], f32, name="gr", tag="gr", bufs=2)
                              for h in range(2)]
                        for h in range(2):
                            nc.scalar.dma_start(
                                gr[h][:], gR[(2 * p + h) * D:(2 * p + h + 1) * D,
                                             t0:t0 + 512])
                        yc = [ycp.tile([D + 1, 512], f32, name=f"yc{h}", tag=f"yc{h}")
                              for h in range(2)]
                        n_loc = 4 * j + 4    # local s-tiles covering this strip

                        def stile(si, first, last, masked):
                            off = max(0, 128 * si - t0) if masked else 0
                            n = 512 - off
                            diag = masked and 128 * si >= t0
                            psc = pscp.tile([128, 1024], f32, name="psc", tag="psc")
                            for h in range(2):
                                nc.tensor.matmul(
                                    psc[:, 512 * h + off:512 * h + 512],
                                    mmc(kt_sb[p][64 * h:64 * h + 64,
                                                 128 * si:128 * si + 128]),
                                    mmc(qt_sb[p][64 * h:64 * h + 64,
                                                 t0 + off:t0 + 512]),
                                    tile_position=(64 * h, 0),
                                    start=True, stop=not diag)
                            if diag:
                                for h in range(2):
                                    nc.tensor.matmul(
                                        psc[:, 512 * h + off:512 * h + off + 128],
                                        mmc(tm_sb[:]), mmc(id_sb[:]),
                                        start=False, stop=True)
                            wt = wtp.tile([128, 1024], SD, name="wt", tag="wt")
                            pin = psc[:].rearrange("p (h t) -> p h t", h=2)[:, :, off:512]
                            wout = wt[:].rearrange("p (h t) -> p h t", h=2)[:, :, off:512]
                            nc.scalar.activation(wout, pin, Exp)
                            for h in range(2):
                                nc.tensor.matmul(
                                    pyt[h][:, off:512],
                                    mmc(v_sb[si][:, (2 * p + h) * (D + 1):
                                                 (2 * p + h + 1) * (D + 1)]),
                                    mmc(wt[:, 512 * h + off:512 * h + 512]),
                                    start=first, stop=last)

                        for si in range(n_loc):
                            stile(si, si == 0, si == n_loc - 1, True)
                        for h in range(2):
                            nc.vector.tensor_copy(yc[h][:], pyt[h][:])
                        for si in range(NLT, NLT + NMT):
                            stile(si, si == NLT, si == NLT + NMT - 1, False)

                        for h in range(2):
                            hh = 2 * p + h
                            den = epp.tile([1, 512], f32, name="den", tag="den", bufs=1)
                            rec = epp.tile([1, 512], SD, name="rec", tag="rec", bufs=2)
                            tmp = epp.tile([D, 512], f32, name="tmp", tag="tmp")
                            nc.vector.tensor_add(den[:], yc[h][D:D + 1, :],
                                                 pyt[h][D:D + 1, :])
                            with nc.allow_low_precision(
                                    reason="softmax denom reciprocal rounded to matmul dtype"):
                                nc.vector.reciprocal(rec[:], den[:])
                            recp = pproj.tile([D, 512], f32, name="recp", tag="pproj")
                            nc.tensor.matmul(recp[:], mmc(oc_sb[:]), mmc(rec[:]),
                                             start=True, stop=True)
                            nc.vector.tensor_mul(
                                tmp[:], pyt[h][0:D, :],
                                gr[h][:])
                            nc.vector.tensor_add(tmp[:], tmp[:], yc[h][0:D, :])
                            nc.vector.tensor_mul(
                                yt_sb[p][64 * h:64 * h + 64, t0:t0 + 512],
                                tmp[:], recp[:])

                for j in range(NJ):
                    for p in range(2):
                        do_strip(p, j)
                    # out-proj for this strip's four 128-row t-tiles
                    for tt in range(4 * j, 4 * j + 4):
                        for cc2 in range(C // W3):
                            po = pproj.tile([128, W3], f32, name="pproj", tag="pproj")
                            for p in range(2):
                                nc.tensor.matmul(
                                    po[:],
                                    mmc(yt_sb[p][:, 128 * tt:128 * tt + 128]),
                                    mmc(wo_sb[p][:, W3 * cc2:W3 * cc2 + W3]),
                                    start=(p == 0), stop=(p == 1))
                            ot = osp.tile([128, W3], f32, name="ot", tag="ot")
                            nc.vector.tensor_copy(ot[:], po[:])
                            nc.sync.dma_start(
                                outp[128 * tt:128 * tt + 128,
                                     W3 * cc2:W3 * cc2 + W3], ot[:])

            if reps > 1:
                with tc.For_i(0, reps, 1) as iv:
                    body(iv)
            else:
                body()

    nc.compile()
    return nc


def get_nc(mm_dtype="float32r", reps=1):
    key = (mm_dtype, reps)
    if key not in _CACHE:
        _CACHE[key] = _build(mm_dtype, reps)
    return _CACHE[key]


# ------------------------------------------------------------ host side

def _prep_inputs(x, forward_memory, reverse_memory, Wq, Wk, Wv, Wo,
                 gate_W, gate_b, mm_dtype="float32r"):
    import ml_dtypes
    sd = np.dtype(ml_dtypes.bfloat16) if mm_dtype == "bfloat16" else np.float32
    f32 = np.float32
    scale = f32(1.0) / f32(np.sqrt(D).astype(np.float32))

    # gate path on host: g = sigmoid(x @ (Wq @ gate_W) + gate_b)
    wg_eff = (Wq @ gate_W).astype(f32)                    # [C, H]
    glog = x.reshape(-1, C) @ wg_eff + gate_b             # [B*T, H]
    g = 1.0 / (1.0 + np.exp(-glog.astype(f32)))           # [B*T, H]
    g = g.reshape(B, T, H)

    bft = np.dtype(ml_dtypes.bfloat16)
    tri = np.triu(np.full((128, 128), -1e30, np.float32), 1)
    tri = np.ascontiguousarray(tri.astype(bft))
    idn = np.ascontiguousarray(np.eye(128, dtype=np.float32).astype(bft))

    in_maps = []
    for c in range(N_CORES):
        b, hg = c // (N_CORES // B), c % (N_CORES // B)
        hs = slice(hg * CC, (hg + 1) * CC)
        kvt = np.concatenate(
            [x[b], forward_memory[b], reverse_memory[b]], axis=0).T
        in_maps.append({
            "kvT": np.ascontiguousarray(kvt.astype(sd)),
            "wq": np.ascontiguousarray((Wq[:, hs] * scale).astype(sd)),
            "wk": np.ascontiguousarray(Wk[:, hs].astype(sd)),
            "wv": np.ascontiguousarray(Wv[:, hs].astype(sd)),
            "wo": np.ascontiguousarray(Wo[hs, :].astype(sd)),
            "gR": np.ascontiguousarray(np.repeat(
                g[b, :, hg * HL:(hg + 1) * HL].T.astype(f32), D, axis=0)),
            "onec": np.ones((1, 64), sd),
            "trineg": tri,
            "ident": idn,
        })
    loss = np.float32(GATE_REG_STRENGTH) * np.mean(g, dtype=np.float32)
    return in_maps, loss


def kernel(x, forward_memory, reverse_memory, Wq, Wk, Wv, Wo, gate_W, gate_b,
           mm_dtype="float32r"):
    from concourse.bass_utils import run_bass_kernel_spmd

    args = [np.asarray(a, np.float32) for a in
            (x, forward_memory, reverse_memory, Wq, Wk, Wv, Wo,
             gate_W, gate_b)]
    in_maps, loss = _prep_inputs(*args, mm_dtype=mm_dtype)
    nc = get_nc(mm_dtype=mm_dtype, reps=1)
    res = run_bass_kernel_spmd(nc, in_maps, list(range(N_CORES)))
    out = np.zeros((B, T, C), np.float32)
    for c in range(N_CORES):
        out[c // (N_CORES // B)] += res.results[c]["outp"]
    return out, np.float32(loss)


# revision 33
# speedup vs baseline: 2.9867x; 2.9867x over previous
"""CMA (chunked-memory attention) Trainium2 kernel.

Sharding: 8 cores = batch(2) x head-groups(4). Each core computes 4 heads
of one batch element end-to-end (q/k/v projections, attention, and its
partial contribution Y_hs @ Wo[hs,:] to the output projection). The host
pre-transposes inputs into [C, S] layout, folds the 1/sqrt(D) attention
scale into Wq, evaluates the tiny gate path g = sigmoid(x @ (Wq@gate_W) + b)
(0.1% of FLOPs), and sum-reduces the four per-head-group output partials
per batch at unshard time (the reduction implied by splitting Wo rows).

Per-core kernel structure:
  phase 1: stream kvT column blocks [C, 512]; project kT/qT (transposed,
           head-channel on partitions) and v (s on partitions, with a
           fused ones-column per head for the softmax denominator).
  phase 2: per head-pair, per 512-wide t strip: QK^T with both heads
           row-packed in the 128x128 PE array (K=64 each), exp straight
           out of PSUM on ScalarE (scores are in [-3, 3], so no
           max-subtraction is needed), causal handling via partial-column
           matmuls plus one triangular [128,128] mask on diagonal blocks,
           then w^T @ [v | 1] accumulating numerator and denominator in
           one PSUM tile. Local (causal chunk) and memory key ranges use
           the same accumulator sequentially; the gate blends them in a
           small DVE epilogue that also normalizes.
  phase 3: out = Y^T.T @ Wo[hs,:] accumulated over the two 128-row
           head-channel tiles.
"""

import numpy as np

# ---------------------------------------------------------------- config

N_CORES = 8
B, T, MEM, C, H = 2, 2048, 512, 1024, 16
SM = 2 * MEM          # total memory keys
S = T + SM            # total keys
HL = H // (N_CORES // B)   # 4 local heads per core
D = C // H            # 64
CC = HL * D           # 256 local head channels
NK = C // 128         # contraction k-tiles
CB = 512              # kvT column-block width
NB = S // CB          # 6 column blocks
NJ = T // 512         # 4 t-strips
NLT = T // 128        # 16 local s-tiles
NMT = SM // 128       # 8 memory s-tiles

GATE_REG_STRENGTH = 0.01

_CACHE = {}


# ------------------------------------------------------------ bass build

def _build(mm_dtype="float32r", reps=1, parts="all"):
    """Build + compile the per-core Bass program (identical on all cores)."""
    import concourse.bass as bass
    import concourse.tile as tile
    from concourse import bacc, mybir

    f32 = mybir.dt.float32
    bf16 = mybir.dt.bfloat16
    if mm_dtype == "bfloat16":
        SD = bf16          # storage dtype for matmul operands
    elif mm_dtype == "float32r":
        # walrus requires fp32r matmul operands to be *produced* as fp32r
        # (the writing engine rounds), so the whole storage path is fp32r.
        SD = mybir.dt.float32r
    else:
        SD = f32

    def mmc(ap):
        return ap

    nc = bacc.Bacc("TRN2", target_bir_lowering=False, debug=False,
                   num_devices=N_CORES)

    kvT = nc.dram_tensor("kvT", [C, S], SD, kind="ExternalInput").ap()
    wq = nc.dram_tensor("wq", [C, CC], SD, kind="ExternalInput").ap()
    wk = nc.dram_tensor("wk", [C, CC], SD, kind="ExternalInput").ap()
    wv = nc.dram_tensor("wv", [C, CC], SD, kind="ExternalInput").ap()
    wo = nc.dram_tensor("wo", [CC, C], SD, kind="ExternalInput").ap()
    gR = nc.dram_tensor("gR", [HL * D, T], f32, kind="ExternalInput").ap()
    onec = nc.dram_tensor("onec", [1, 64], SD, kind="ExternalInput").ap()
    trineg = nc.dram_tensor("trineg", [128, 128], bf16,
                            kind="ExternalInput").ap()
    ident = nc.dram_tensor("ident", [128, 128], bf16,
                           kind="ExternalInput").ap()
    outp = nc.dram_tensor("outp", [T, C], f32, kind="ExternalOutput").ap()

    Exp = mybir.ActivationFunctionType.Exp

    with tile.TileContext(nc) as tc:
        from contextlib import ExitStack
        with ExitStack() as ctx:
            const = ctx.enter_context(tc.tile_pool(name="const", bufs=1))
            kvp = ctx.enter_context(tc.tile_pool(name="kvp", bufs=2))
            qtp = ctx.enter_context(tc.tile_pool(name="qtp", bufs=1))
            ktp = ctx.enter_context(tc.tile_pool(name="ktp", bufs=1))
            vp = ctx.enter_context(tc.tile_pool(name="vp", bufs=1))
            ytp = ctx.enter_context(tc.tile_pool(name="ytp", bufs=1))
            wtp = ctx.enter_context(tc.tile_pool(name="wtp", bufs=3))
            ycp = ctx.enter_context(tc.tile_pool(name="ycp", bufs=1))
            epp = ctx.enter_context(tc.tile_pool(name="epp", bufs=2))
            osp = ctx.enter_context(tc.tile_pool(name="osp", bufs=2))
            pproj = ctx.enter_context(
                tc.tile_pool(name="pproj", bufs=2, space="PSUM"))
            pscp = ctx.enter_context(
                tc.tile_pool(name="pscp", bufs=2, space="PSUM"))
            pytp = ctx.enter_context(
                tc.tile_pool(name="pytp", bufs=2, space="PSUM"))

            # ---- constants (loaded once, outside the reps loop) ----
            wq_sb = [const.tile([128, CC], SD, name=f"wq{k}", tag=f"wq{k}") for k in range(NK)]
            wk_sb = [const.tile([128, CC], SD, name=f"wk{k}", tag=f"wk{k}") for k in range(NK)]
            wv_sb = [const.tile([128, CC], SD, name=f"wv{k}", tag=f"wv{k}") for k in range(NK)]
            wo_sb = [const.tile([128, C], SD, name=f"wo{k}", tag=f"wo{k}") for k in range(CC // 128)]
            tm_sb = const.tile([128, 128], bf16, name="tm", tag="tm")
            id_sb = const.tile([128, 128], bf16, name="idn", tag="idn")
            oc_sb = const.tile([1, 64], SD, name="oc", tag="oc")
            nc.scalar.dma_start(oc_sb[:], onec[:])
            for k in range(NK):
                nc.scalar.dma_start(wk_sb[k][:], wk[128 * k:128 * k + 128, :])
            for k in range(NK):
                nc.scalar.dma_start(wq_sb[k][:], wq[128 * k:128 * k + 128, :])
            for k in range(NK):
                nc.scalar.dma_start(wv_sb[k][:], wv[128 * k:128 * k + 128, :])
            for k in range(CC // 128):
                nc.scalar.dma_start(wo_sb[k][:], wo[128 * k:128 * k + 128, :])
            nc.scalar.dma_start(tm_sb[:], trineg[:])
            nc.scalar.dma_start(id_sb[:], ident[:])
            # warm the Exp table while phase 1 runs
            warm = const.tile([1, 1], f32, name="warm", tag="warm")
            nc.vector.memset(warm[:], 0.0)
            nc.scalar.activation(warm[:], warm[:], Exp)
            ones4 = const.tile([128, HL], f32, name="ones4", tag="ones4")
            nc.vector.memset(ones4[:], 1.0)

            def body(_iv=None, parts="all"):
                # persistent per-iteration outputs
                qt_sb = [qtp.tile([128, T], SD, name=f"qt{m}", tag=f"qt{m}") for m in range(2)]
                kt_sb = [ktp.tile([128, S], SD, name=f"kt{m}", tag=f"kt{m}") for m in range(2)]
                v_sb = [vp.tile([128, HL * (D + 1)], SD, name=f"v{si}", tag=f"v{si}")
                        for si in range(S // 128)]
                yt_sb = [ytp.tile([128, T], SD, name=f"yt{p}", tag=f"yt{p}") for p in range(2)]

                # ---------------- phase 1: projections ----------------
                for b in (range(NB) if parts in ("all", "p1") else range(0)):
                    c0 = CB * b
                    kvt = [kvp.tile([128, CB], SD, name=f"kv{k}", tag=f"kv{k}")
                           for k in range(NK)]
                    for k in range(NK):
                        eng = nc.sync if k % 2 == 0 else nc.scalar
                        eng.dma_start(
                            kvt[k][:], kvT[128 * k:128 * k + 128, c0:c0 + CB])
                    # kT (and qT for local columns): [cout 128, cols]
                    for m in range(2):
                        pk = pproj.tile([128, CB], f32, name="pproj", tag="pproj")
                        for k in range(NK):
                            nc.tensor.matmul(
                                pk[:], mmc(wk_sb[k][:, 128 * m:128 * m + 128]),
                                mmc(kvt[k][:]), start=(k == 0), stop=(k == NK - 1))
                        nc.vector.tensor_copy(kt_sb[m][:, c0:c0 + CB], pk[:])
                    if c0 < T:
                        for m in range(2):
                            pq = pproj.tile([128, CB], f32, name="pproj", tag="pproj")
                            for k in range(NK):
                                nc.tensor.matmul(
                                    pq[:], mmc(wq_sb[k][:, 128 * m:128 * m + 128]),
                                    mmc(kvt[k][:]), start=(k == 0),
                                    stop=(k == NK - 1))
                            nc.vector.tensor_copy(qt_sb[m][:, c0:c0 + CB], pq[:])
                    # v: [s 128, cout CC] -> packed [s, HL*(D+1)] with ones col
                    for sv in range(CB // 128):
                        si = (CB * b) // 128 + sv
                        pv = pproj.tile([128, CC], f32, name="pproj", tag="pproj")
                        for k in range(NK):
                            nc.tensor.matmul(
                                pv[:],
                                mmc(kvt[k][:, 128 * sv:128 * sv + 128]),
                                mmc(wv_sb[k][:]), start=(k == 0),
                                stop=(k == NK - 1))
                        vt = v_sb[si]
                        # strided copy: head h -> cols [h*(D+1), h*(D+1)+D)
                        dst = vt[:].rearrange("p (h e) -> p h e", h=HL)[:, :, 0:D]
                        src = pv[:].rearrange("p (h d) -> p h d", h=HL)
                        nc.vector.tensor_copy(dst, src)
                        ones = vt[:].rearrange("p (h e) -> p h e", h=HL)[:, :, D:D + 1]
                        nc.vector.tensor_copy(
                            ones, ones4[:].unsqueeze(2))

                # -------- phase 2: attention + interleaved out-proj --------
                if parts == "p1":
                    return
                W3 = min(512, C)

                def do_strip(p, j):
                        t0 = 512 * j
                        pyt = [pytp.tile([D + 1, 512], f32, name="pyt", tag="pyt")
                               for h in range(2)]
                        gr = [epp.tile([D, 512], f32, name="gr", tag="gr", bufs=2)
                              for h in range(2)]
                        for h in range(2):
                            nc.scalar.dma_start(
                                gr[h][:], gR[(2 * p + h) * D:(2 * p + h + 1) * D,
                                             t0:t0 + 512])
                        yc = [ycp.tile([D + 1, 512], f32, name=f"yc{h}", tag=f"yc{h}")
                              for h in range(2)]
                        n_loc = 4 * j + 4    # local s-tiles covering this strip

                        def sc_exp(si, masked):
                            """QK^T (pair row-packed) + causal bias + exp.
                            Returns (wt, off) for the delayed PV step."""
                            off = max(0, 128 * si - t0) if masked else 0
                            diag = masked and 128 * si >= t0
                            psc = pscp.tile([128, 1024], f32, name="psc", tag="psc")
                            for h in range(2):
                                nc.tensor.matmul(
                                    psc[:, 512 * h + off:512 * h + 512],
                                    mmc(kt_sb[p][64 * h:64 * h + 64,
                                                 128 * si:128 * si + 128]),
                                    mmc(qt_sb[p][64 * h:64 * h + 64,
                                                 t0 + off:t0 + 512]),
                                    tile_position=(64 * h, 0),
                                    start=True, stop=not diag)
                            if diag:
                                for h in range(2):
                                    nc.tensor.matmul(
                                        psc[:, 512 * h + off:512 * h + off + 128],
                                        mmc(tm_sb[:]), mmc(id_sb[:]),
                                        start=False, stop=True)
                            wt = wtp.tile([128, 1024], SD, name="wt", tag="wt")
                            pin = psc[:].rearrange("p (h t) -> p h t", h=2)[:, :, off:512]
                            wout = wt[:].rearrange("p (h t) -> p h t", h=2)[:, :, off:512]
                            nc.scalar.activation(wout, pin, Exp)
                            return wt, off

                        def pv(si, wt, off, first, last):
                            for h in range(2):
                                nc.tensor.matmul(
                                    pyt[h][:, off:512],
                                    mmc(v_sb[si][:, (2 * p + h) * (D + 1):
                                                 (2 * p + h + 1) * (D + 1)]),
                                    mmc(wt[:, 512 * h + off:512 * h + 512]),
                                    start=first, stop=last)

                        # software pipeline: PV trails sc/exp by one unit so the
                        # PE stream never blocks on the ACT exp of the same unit
                        units = ([(si, True) for si in range(n_loc)] +
                                 [(si, False) for si in range(NLT, NLT + NMT)])
                        pend = None
                        for u_si, u_masked in units:
                            wt_off = sc_exp(u_si, u_masked)
                            if pend is not None:
                                s_si, s_wt, s_off = pend
                                pv(s_si, s_wt, s_off, s_si in (0, NLT),
                                   s_si in (n_loc - 1, NLT + NMT - 1))
                                if s_si == n_loc - 1:
                                    for h in range(2):
                                        nc.vector.tensor_copy(yc[h][:], pyt[h][:])
                            pend = (u_si, wt_off[0], wt_off[1])
                        pv(pend[0], pend[1], pend[2], pend[0] in (0, NLT),
                           pend[0] in (n_loc - 1, NLT + NMT - 1))

                        for h in range(2):
                            hh = 2 * p + h
                            den = epp.tile([1, 512], f32, name="den", tag="den", bufs=1)
                            rec = epp.tile([1, 512], SD, name="rec", tag="rec", bufs=2)
                            tmp = epp.tile([D, 512], f32, name="tmp", tag="tmp")
                            nc.vector.tensor_add(den[:], yc[h][D:D + 1, :],
                                                 pyt[h][D:D + 1, :])
                            with nc.allow_low_precision(
                                    reason="softmax denom reciprocal rounded to matmul dtype"):
                                nc.vector.reciprocal(rec[:], den[:])
                            recp = pproj.tile([D, 512], f32, name="recp", tag="pproj")
                            nc.tensor.matmul(recp[:], mmc(oc_sb[:]), mmc(rec[:]),
                                             start=True, stop=True)
                            nc.vector.tensor_mul(
                                tmp[:], pyt[h][0:D, :],
                                gr[h][:])
                            nc.vector.tensor_add(tmp[:], tmp[:], yc[h][0:D, :])
                            nc.vector.tensor_mul(
                                yt_sb[p][64 * h:64 * h + 64, t0:t0 + 512],
                                tmp[:], recp[:])

                for j in range(NJ):
                    for p in range(2):
                        do_strip(p, j)
                    # out-proj for this strip's four 128-row t-tiles
                    for tt in range(4 * j, 4 * j + 4):
                        for cc2 in range(C // W3):
                            po = pproj.tile([128, W3], f32, name="pproj", tag="pproj")
                            for p in range(2):
                                nc.tensor.matmul(
                                    po[:],
                                    mmc(yt_sb[p][:, 128 * tt:128 * tt + 128]),
                                    mmc(wo_sb[p][:, W3 * cc2:W3 * cc2 + W3]),
                                    start=(p == 0), stop=(p == 1))
                            ot = osp.tile([128, W3], f32, name="ot", tag="ot")
                            nc.vector.tensor_copy(ot[:], po[:])
                            nc.sync.dma_start(
                                outp[128 * tt:128 * tt + 128,
                                     W3 * cc2:W3 * cc2 + W3], ot[:])

            if parts == "p23" and reps > 1:
                body(parts="p1")        # projections once, outside the loop
                with tc.For_i(0, reps, 1) as iv:
                    body(iv, parts="p23")
            elif reps > 1:
                with tc.For_i(0, reps, 1) as iv:
                    body(iv, parts=parts)
            else:
                body(parts=parts)

    nc.compile()
    return nc


def get_nc(mm_dtype="float32r", reps=1, parts="all"):
    key = (mm_dtype, reps, parts)
    if key not in _CACHE:
        _CACHE[key] = _build(mm_dtype, reps, parts)
    return _CACHE[key]


# ------------------------------------------------------------ host side

def _prep_inputs(x, forward_memory, reverse_memory, Wq, Wk, Wv, Wo,
                 gate_W, gate_b, mm_dtype="float32r"):
    import ml_dtypes
    sd = np.dtype(ml_dtypes.bfloat16) if mm_dtype == "bfloat16" else np.float32
    f32 = np.float32
    scale = f32(1.0) / f32(np.sqrt(D).astype(np.float32))

    # gate path on host: g = sigmoid(x @ (Wq @ gate_W) + gate_b)
    wg_eff = (Wq @ gate_W).astype(f32)                    # [C, H]
    glog = x.reshape(-1, C) @ wg_eff + gate_b             # [B*T, H]
    g = 1.0 / (1.0 + np.exp(-glog.astype(f32)))           # [B*T, H]
    g = g.reshape(B, T, H)

    bft = np.dtype(ml_dtypes.bfloat16)
    tri = np.triu(np.full((128, 128), -1e30, np.float32), 1)
    tri = np.ascontiguousarray(tri.astype(bft))
    idn = np.ascontiguousarray(np.eye(128, dtype=np.float32).astype(bft))

    in_maps = []
    for c in range(N_CORES):
        b, hg = c // (N_CORES // B), c % (N_CORES // B)
        hs = slice(hg * CC, (hg + 1) * CC)
        kvt = np.concatenate(
            [x[b], forward_memory[b], reverse_memory[b]], axis=0).T
        in_maps.append({
            "kvT": np.ascontiguousarray(kvt.astype(sd)),
            "wq": np.ascontiguousarray((Wq[:, hs] * scale).astype(sd)),
            "wk": np.ascontiguousarray(Wk[:, hs].astype(sd)),
            "wv": np.ascontiguousarray(Wv[:, hs].astype(sd)),
            "wo": np.ascontiguousarray(Wo[hs, :].astype(sd)),
            "gR": np.ascontiguousarray(np.repeat(
                g[b, :, hg * HL:(hg + 1) * HL].T.astype(f32), D, axis=0)),
            "onec": np.ones((1, 64), sd),
            "trineg": tri,
            "ident": idn,
        })
    loss = np.float32(GATE_REG_STRENGTH) * np.mean(g, dtype=np.float32)
    return in_maps, loss


def kernel(x, forward_memory, reverse_memory, Wq, Wk, Wv, Wo, gate_W, gate_b,
           mm_dtype="float32r"):
    from concourse.bass_utils import run_bass_kernel_spmd

    args = [np.asarray(a, np.float32) for a in
            (x, forward_memory, reverse_memory, Wq, Wk, Wv, Wo,
             gate_W, gate_b)]
    in_maps, loss = _prep_inputs(*args, mm_dtype=mm_dtype)
    nc = get_nc(mm_dtype=mm_dtype, reps=1)
    res = run_bass_kernel_spmd(nc, in_maps, list(range(N_CORES)))
    out = np.zeros((B, T, C), np.float32)
    for c in range(N_CORES):
        out[c // (N_CORES // B)] += res.results[c]["outp"]
    return out, np.float32(loss)
